# revision 25
# baseline (speedup 1.0000x reference)
"""AstrocyteGate distributed Bass kernel for one TRN2 chip (8 NeuronCores).

Reference computation (B=8, T=2048, D=2048, fp32):
    pooled    = mean over (B*T) of x            -> [D]
    update    = proj_w @ pooled + proj_b        -> [D]
    new_state = DECAY*state + (1-DECAY)*update  -> [D]
    gain      = sigmoid(gate_w @ new_state + gate_b)
    out       = x * gain                        (broadcast over [B,T,D])

Strategy (data-parallel over B, 1 batch row per core, x TRANSPOSED + bf16):
  - The host ships x[c] as bf16 in [D, T] layout, tiled [4, 128, 4, 2048]
    (partition = d within a 128-chunk, 2 MiB per DMA). 8 MiB in, 8 MiB
    out per core -- half the fp32 traffic; bf16 rounding (~1e-3 rel) is
    far inside the 2e-2 gate, and the pooled-mean contribution to the
    gate is damped by (1-DECAY)=1e-3.
  - In this layout the token-sum is a free-axis vector reduce straight
    into sT[128, 16], the gate logit lives in [128, 16] transposed form
    end-to-end (sigmoid on 16 columns, no [128, 2048] broadcast), and
    the final scaling is a per-partition tensor_scalar at 2x bf16 DVE
    rate (0.66us per [128, 2048] slice).
  - A zero-dependency warm-up AllGather issues first: the ncfw cold
    path + cross-core launch rendezvous costs 60-80us and dominates the
    critical path; the x load, token-sums and weight loads all hide
    under it. exec ~= warmup_done + AG1 + matvec chain + AG2 + stores.
  - AllGather #1 shares the 8 local token-sums ([128, 16] fp32, 8 KB,
    one gather-load DMA via a rank-transposed AP); one free-axis reduce
    over a transposed SBUF view sums the ranks. The two DxD matvecs are
    sharded 256 rows/columns per core as (LDWEIGHTS + N=1 MATMUL) pairs
    (~27 ns each, 32 pairs per matvec). All affine constants fold on
    the host: (1-DECAY)/(B*T) into proj_w, and the x-independent logit
    gate_w @ (DECAY*state + (1-DECAY)*proj_b) + gate_b into a single
    bias (split /8 so the AG2 rank-sum restores it), so the device EMA
    is just a PSUM->bf16 copy.
  - AllGather #2 shares the 8 partial gate logits; rank-sum reduce +
    sigmoid yields gainT[128, 16]; x tiles are scaled in place and
    streamed out as 2 MiB stores alternating both HWDGE rings.

x is read from HBM exactly once and out written once (8+8 MiB bf16 per
core) -> ~50us of DMA at the ~330 GB/s realized rate, of which the load
half is hidden under the warm-up collective window.
"""

import numpy as np

import concourse.bacc as bacc
import concourse.bass as bass
import concourse.mybir as mybir
import concourse.tile as tile
from concourse.bass_utils import run_bass_kernel_spmd

B, T, D = 8, 2048, 2048
NCORES = 8
DS = D // NCORES        # 256: per-core shard of D (matvec rows/cols)
NT = 4                  # macro-tiles of x per core (2 MiB each)
TAU = 1000.0
DECAY = float(np.exp(-1.0 / TAU))
A = 1.0 - DECAY
FP32 = mybir.dt.float32
BF16 = mybir.dt.bfloat16
RG = [list(range(NCORES))]

_NC_CACHE = {}


def _build():
    nc = bacc.Bacc(
        "TRN2",
        target_bir_lowering=False,
        debug=False,
        enable_asserts=False,
        num_devices=NCORES,
    )

    xt_d = nc.dram_tensor("xt", [NT, 128, 4, T], BF16, kind="ExternalInput")
    ptw_d = nc.dram_tensor("ptw", [128, 16, 16, 128], BF16, kind="ExternalInput")
    gtw_d = nc.dram_tensor("gtw", [128, 16, 16, 128], BF16, kind="ExternalInput")
    gbt_d = nc.dram_tensor("gbt", [128, 16], FP32, kind="ExternalInput")
    out_d = nc.dram_tensor("out", [NT, 128, 4, T], BF16, kind="ExternalOutput")

    wsync_in = nc.dram_tensor("wsync_in", [1, 16], FP32)
    wsync_out = nc.dram_tensor("wsync_out", [1, 16], FP32, addr_space="Shared")
    u_bnc = nc.dram_tensor("u_bnc", [128, 16], FP32)
    ar_out = nc.dram_tensor("ar_out", [128, 16], FP32, addr_space="Shared")

    AF = mybir.ActivationFunctionType
    ALU = mybir.AluOpType
    AX = mybir.AxisListType

    with tile.TileContext(nc) as tc:
        with (
            tc.tile_pool(name="xpool", bufs=NT) as xpool,
            tc.tile_pool(name="wpool", bufs=1) as wpool,
            tc.tile_pool(name="small", bufs=1) as small,
            tc.tile_pool(name="psA", bufs=1, space="PSUM") as psA,
            tc.tile_pool(name="psB", bufs=1, space="PSUM") as psB,
        ):
            # --- warm-up collective: ncfw wake + rank rendezvous, no deps.
            # AllReduce so the CCE/reduce SPAD path is the one warmed. ---
            nc.gpsimd.collective_compute(
                "AllReduce",
                ALU.add,
                replica_groups=RG,
                ins=[wsync_in.ap().opt()],
                outs=[wsync_out.ap().opt()],
            )

            # --- load x first; everything else is off the critical path ---
            xs = []
            for j in range(NT):
                xtile = xpool.tile([128, 4, T], BF16, tag="xt")
                nc.sync.dma_start(xtile[:], xt_d[j])
                xs.append(xtile)

            # pre-warm the ScalarE sigmoid LUT off the critical path
            dummy = small.tile([1, 1], FP32, tag="dummy")
            nc.vector.memset(dummy[:], 1.0)
            nc.scalar.activation(dummy[:], dummy[:], AF.Sigmoid)

            # --- weight / bias loads (off the critical path) ---
            ptw = wpool.tile([128, 16, 16, 128], BF16, tag="ptw")
            nc.sync.dma_start(ptw[:], ptw_d[:])
            gtw = wpool.tile([128, 16, 16, 128], BF16, tag="gtw")
            nc.sync.dma_start(gtw[:], gtw_d[:])
            gbT = small.tile([128, 16], FP32, tag="gbt")
            nc.sync.dma_start(gbT[:], gbt_d[:])

            # --- local token-sums: free-axis reduce per tile as it lands ---
            sT = small.tile([128, 16], FP32, tag="sT")
            for j in range(NT):
                nc.vector.tensor_reduce(
                    sT[:, 4 * j : 4 * j + 4], xs[j][:], axis=AX.X, op=ALU.add
                )
            sTb = small.tile([128, 16], BF16, tag="sTb")
            nc.vector.tensor_copy(sTb[:], sT[:])

            # --- matvec1 on LOCAL sums, hidden under the warm-up window:
            #     u_c = ((1-DECAY)*proj_w/(B*T)) @ s_c  (full DxD, 256 pairs)
            u_ps = psA.tile([128, 16], FP32, tag="upd")
            for m in range(16):
                for kk in range(16):
                    nc.tensor.matmul(
                        u_ps[:, m : m + 1],
                        ptw[:, m, kk, :],
                        sTb[:, kk : kk + 1],
                        start=(kk == 0),
                        stop=(kk == 15),
                    )
            u_sb = small.tile([128, 16], FP32, tag="u_sb")
            nc.vector.tensor_copy(u_sb[:], u_ps[:])
            nc.sync.dma_start(u_bnc[:], u_sb[:], single_packet=True)

            # --- ONE AllReduce: sum_c u_c = ns (CCE adds on the wire) ---
            nc.gpsimd.collective_compute(
                "AllReduce",
                ALU.add,
                replica_groups=RG,
                ins=[u_bnc.ap().opt()],
                outs=[ar_out.ap().opt()],
            )
            nsF = small.tile([128, 16], FP32, tag="nsF")
            nc.sync.dma_start(nsF[:], ar_out[:], single_packet=True)
            nsT = small.tile([128, 16], BF16, tag="nsT")
            nc.vector.tensor_copy(nsT[:], nsF[:])

            # --- matvec2 full-local: logitT = gate_w @ ns (256 pairs) ---
            logit_ps = psB.tile([128, 16], FP32, tag="logit")
            for kk in range(16):
                for jj in range(16):
                    nc.tensor.matmul(
                        logit_ps[:, jj : jj + 1],
                        gtw[:, kk, jj, :],
                        nsT[:, kk : kk + 1],
                        start=(kk == 0),
                        stop=(kk == 15),
                    )
            lsum = small.tile([128, 16], FP32, tag="lsum")
            nc.vector.tensor_add(lsum[:], logit_ps[:], gbT[:])
            gainT = small.tile([128, 16], FP32, tag="gainT")
            nc.scalar.activation(gainT[:], lsum[:], AF.Sigmoid)

            # --- scale x in place (per-partition scalar) and stream out ---
            for j in range(NT):
                for q in range(4):
                    nc.vector.tensor_scalar_mul(
                        xs[j][:, q, :],
                        xs[j][:, q, :],
                        gainT[:, 4 * j + q : 4 * j + q + 1],
                    )
                    # first tile: store in halves so the write stream starts
                    # as early as possible after gain is known
                    if j == 0 and q == 1:
                        nc.sync.dma_start(out_d[j, :, 0:2, :], xs[j][:, 0:2, :])
                if j == 0:
                    nc.scalar.dma_start(out_d[j, :, 2:4, :], xs[j][:, 2:4, :])
                else:
                    # alternate the two HWDGE rings to keep all SDMA slots fed
                    eng = nc.sync if j % 2 == 0 else nc.scalar
                    eng.dma_start(out_d[j], xs[j][:])

    nc.compile()
    return nc


def _get_nc():
    if "nc" not in _NC_CACHE:
        _NC_CACHE["nc"] = _build()
    return _NC_CACHE["nc"]


def _shard_inputs(x, state, proj_w, proj_b, gate_w, gate_b):
    import ml_dtypes

    bf16 = ml_dtypes.bfloat16
    x = np.asarray(x, dtype=np.float32)
    state = np.asarray(state, dtype=np.float32)
    proj_w = np.asarray(proj_w, dtype=np.float32)
    proj_b = np.asarray(proj_b, dtype=np.float32)
    gate_w = np.asarray(gate_w, dtype=np.float32)
    gate_b = np.asarray(gate_b, dtype=np.float32)

    # EMA affine constant: ns = A*(proj_w/(B*T)) @ pooled_sum + eb, so the
    # x-independent gate logit part gate_w @ eb + gate_b folds into one bias.
    eb = DECAY * state + A * proj_b
    gb_eff = gate_w @ eb + gate_b
    gbt = np.ascontiguousarray(gb_eff.reshape(16, 128).T)
    # full (replicated) weights: ptw[p, m, kk, j] = (1-DECAY)*proj_w[128m+j, 128kk+p]/(B*T)
    ptw = np.ascontiguousarray(
        (proj_w * (A / float(B * T)))
        .reshape(16, 128, 16, 128)
        .transpose(3, 0, 2, 1)
        .astype(bf16)
    )
    # gtw[p, kk, jj, m] = gate_w[128jj+m, 128kk+p]
    gtw = np.ascontiguousarray(
        gate_w.reshape(16, 128, 16, 128).transpose(3, 2, 0, 1).astype(bf16)
    )
    in_maps = []
    for c in range(NCORES):
        # xt[J, p, q, t] = x[c, t, 128*(4J+q)+p], bf16
        xt = np.ascontiguousarray(
            x[c].T.reshape(NT, 4, 128, T).transpose(0, 2, 1, 3).astype(bf16)
        )
        in_maps.append({"xt": xt, "ptw": ptw, "gtw": gtw, "gbt": gbt})
    return in_maps


def _run(inputs, trace=False, **kwargs):
    nc = _get_nc()
    in_maps = _shard_inputs(**inputs)
    res = run_bass_kernel_spmd(
        nc, in_maps, core_ids=list(range(NCORES)), trace=trace, **kwargs
    )
    # out[c, t, 128*(2J+q)+p] = out_dev[J, p, q, t]
    outs = []
    for c in range(NCORES):
        od = res.results[c]["out"]  # [NT, 128, 2, T] bf16
        outs.append(
            od.transpose(0, 2, 1, 3).reshape(D, T).T.astype(np.float32)
        )
    out = np.ascontiguousarray(np.stack(outs, axis=0))
    return out, res


def kernel(**inputs):
    out, _ = _run(inputs, trace=False)
    return out


# revision 26
# speedup vs baseline: 1.2665x; 1.2665x over previous
"""AstrocyteGate distributed Bass kernel for one TRN2 chip (8 NeuronCores).

Reference computation (B=8, T=2048, D=2048, fp32):
    pooled    = mean over (B*T) of x            -> [D]
    update    = proj_w @ pooled + proj_b        -> [D]
    new_state = DECAY*state + (1-DECAY)*update  -> [D]
    gain      = sigmoid(gate_w @ new_state + gate_b)
    out       = x * gain                        (broadcast over [B,T,D])

Strategy (data-parallel over B, 1 batch row per core, x TRANSPOSED + bf16):
  - The host ships x[c] as bf16 in [D, T] layout, tiled [4, 128, 4, 2048]
    (partition = d within a 128-chunk, 2 MiB per DMA). 8 MiB in, 8 MiB
    out per core -- half the fp32 traffic; bf16 rounding (~1e-3 rel) is
    far inside the 2e-2 gate, and the pooled-mean contribution to the
    gate is damped by (1-DECAY)=1e-3.
  - In this layout the token-sum is a free-axis vector reduce straight
    into sT[128, 16], the gate logit lives in [128, 16] transposed form
    end-to-end (sigmoid on 16 columns, no [128, 2048] broadcast), and
    the final scaling is a per-partition tensor_scalar at 2x bf16 DVE
    rate (0.66us per [128, 2048] slice).
  - A zero-dependency warm-up AllGather issues first: the ncfw cold
    path + cross-core launch rendezvous costs 60-80us and dominates the
    critical path; the x load, token-sums and weight loads all hide
    under it. exec ~= warmup_done + AG1 + matvec chain + AG2 + stores.
  - AllGather #1 shares the 8 local token-sums ([128, 16] fp32, 8 KB,
    one gather-load DMA via a rank-transposed AP); one free-axis reduce
    over a transposed SBUF view sums the ranks. The two DxD matvecs are
    sharded 256 rows/columns per core as (LDWEIGHTS + N=1 MATMUL) pairs
    (~27 ns each, 32 pairs per matvec). All affine constants fold on
    the host: (1-DECAY)/(B*T) into proj_w, and the x-independent logit
    gate_w @ (DECAY*state + (1-DECAY)*proj_b) + gate_b into a single
    bias (split /8 so the AG2 rank-sum restores it), so the device EMA
    is just a PSUM->bf16 copy.
  - AllGather #2 shares the 8 partial gate logits; rank-sum reduce +
    sigmoid yields gainT[128, 16]; x tiles are scaled in place and
    streamed out as 2 MiB stores alternating both HWDGE rings.

x is read from HBM exactly once and out written once (8+8 MiB bf16 per
core) -> ~50us of DMA at the ~330 GB/s realized rate, of which the load
half is hidden under the warm-up collective window.
"""

import numpy as np

import concourse.bacc as bacc
import concourse.bass as bass
import concourse.mybir as mybir
import concourse.tile as tile
from concourse.bass_utils import run_bass_kernel_spmd

B, T, D = 8, 2048, 2048
NCORES = 8
DS = D // NCORES        # 256: per-core shard of D (matvec rows/cols)
NT = 4                  # macro-tiles of x per core (2 MiB each)
TAU = 1000.0
DECAY = float(np.exp(-1.0 / TAU))
A = 1.0 - DECAY
FP32 = mybir.dt.float32
BF16 = mybir.dt.bfloat16
RG = [list(range(NCORES))]

_NC_CACHE = {}


def _build():
    nc = bacc.Bacc(
        "TRN2",
        target_bir_lowering=False,
        debug=False,
        enable_asserts=False,
        num_devices=NCORES,
    )

    xt_d = nc.dram_tensor("xt", [NT, 128, 4, T], BF16, kind="ExternalInput")
    ptw_d = nc.dram_tensor("ptw", [128, 2, 16, 128], BF16, kind="ExternalInput")
    gtw_d = nc.dram_tensor("gtw", [128, 2, 16, 128], BF16, kind="ExternalInput")
    gbt_d = nc.dram_tensor("gbt", [128, 16], FP32, kind="ExternalInput")
    out_d = nc.dram_tensor("out", [NT, 128, 4, T], BF16, kind="ExternalOutput")

    wsync_in = nc.dram_tensor("wsync_in", [1, 16], BF16)
    wsync_out = nc.dram_tensor("wsync_out", [NCORES, 16], BF16, addr_space="Shared")
    s_bnc = nc.dram_tensor("s_bnc", [128, 16], FP32)
    gath1 = nc.dram_tensor("gath1", [NCORES, 128, 16], FP32, addr_space="Shared")
    l_bnc = nc.dram_tensor("l_bnc", [128, 16], FP32)
    gath2 = nc.dram_tensor("gath2", [NCORES, 128, 16], FP32, addr_space="Shared")

    AF = mybir.ActivationFunctionType
    ALU = mybir.AluOpType
    AX = mybir.AxisListType

    with tile.TileContext(nc) as tc:
        with (
            tc.tile_pool(name="xpool", bufs=NT) as xpool,
            tc.tile_pool(name="wpool", bufs=1) as wpool,
            tc.tile_pool(name="small", bufs=1) as small,
            tc.tile_pool(name="psA", bufs=1, space="PSUM") as psA,
            tc.tile_pool(name="psB", bufs=1, space="PSUM") as psB,
        ):
            # --- warm-up collective: ncfw wake + rank rendezvous, no deps ---
            nc.gpsimd.collective_compute(
                "AllGather",
                ALU.bypass,
                replica_groups=RG,
                ins=[wsync_in.ap().opt()],
                outs=[wsync_out.ap().opt()],
            )

            # --- load x first; everything else is off the critical path ---
            xs = []
            for j in range(NT):
                xtile = xpool.tile([128, 4, T], BF16, tag="xt")
                nc.sync.dma_start(xtile[:], xt_d[j])
                xs.append(xtile)

            # pre-warm the ScalarE sigmoid LUT off the critical path
            dummy = small.tile([1, 1], FP32, tag="dummy")
            nc.vector.memset(dummy[:], 1.0)
            nc.scalar.activation(dummy[:], dummy[:], AF.Sigmoid)

            # --- weight / bias loads (off the critical path) ---
            ptw = wpool.tile([128, 2, 16, 128], BF16, tag="ptw")
            nc.sync.dma_start(ptw[:], ptw_d[:])
            gtw = wpool.tile([128, 2, 16, 128], BF16, tag="gtw")
            nc.sync.dma_start(gtw[:], gtw_d[:])
            gbT = small.tile([128, 16], FP32, tag="gbt")
            nc.sync.dma_start(gbT[:], gbt_d[:])

            # --- local token-sums: free-axis reduce per tile as it lands ---
            sT = small.tile([128, 16], FP32, tag="sT")
            for j in range(NT):
                nc.vector.tensor_reduce(
                    sT[:, 4 * j : 4 * j + 4], xs[j][:], axis=AX.X, op=ALU.add
                )
            nc.sync.dma_start(s_bnc[:], sT[:], single_packet=True)

            # --- AllGather #1: the 8 local sums (fp32, 8 KB each) ---
            nc.gpsimd.collective_compute(
                "AllGather",
                ALU.bypass,
                replica_groups=RG,
                ins=[s_bnc.ap().opt()],
                outs=[gath1.ap().opt()],
            )
            g1 = small.tile([128, NCORES, 16], FP32, tag="g1")
            nc.sync.dma_start(g1[:], gath1.ap().rearrange("r p j -> p r j"), single_packet=True)

            # --- rank-sum via one free-axis reduce over a transposed view ---
            pooled_f = small.tile([128, 16], FP32, tag="pooled_f")
            nc.vector.tensor_reduce(
                pooled_f[:], g1[:].rearrange("p r j -> p j r"), axis=AX.X, op=ALU.add
            )
            pooled = small.tile([128, 16], BF16, tag="pooled")
            nc.vector.tensor_copy(pooled[:], pooled_f[:])

            # --- matvec1: nsT = ((1-DECAY)*proj_w/(B*T))[rows_c, :] @ pooled ---
            # (the EMA decay/bias affine is folded into ptw and gbt host-side)
            upd_ps = psA.tile([128, 2], FP32, tag="upd")
            for m in range(2):
                for kk in range(16):
                    nc.tensor.matmul(
                        upd_ps[:, m : m + 1],
                        ptw[:, m, kk, :],
                        pooled[:, kk : kk + 1],
                        start=(kk == 0),
                        stop=(kk == 15),
                    )
            nsT = small.tile([128, 2], BF16, tag="nsT")
            nc.vector.tensor_copy(nsT[:], upd_ps[:])

            # --- matvec2: partial logitT = gate_w[:, cols_c] @ ns_shard ---
            logit_ps = psB.tile([128, 16], FP32, tag="logit")
            for kk in range(2):
                for jj in range(16):
                    nc.tensor.matmul(
                        logit_ps[:, jj : jj + 1],
                        gtw[:, kk, jj, :],
                        nsT[:, kk : kk + 1],
                        start=(kk == 0),
                        stop=(kk == 1),
                    )
            # fold gate_b/NCORES into each partial so the rank-sum yields +gate_b
            lp = small.tile([128, 16], FP32, tag="lp")
            nc.vector.tensor_add(lp[:], logit_ps[:], gbT[:])
            nc.sync.dma_start(l_bnc[:], lp[:], single_packet=True)

            # --- AllGather #2: the 8 partial logits ---
            nc.gpsimd.collective_compute(
                "AllGather",
                ALU.bypass,
                replica_groups=RG,
                ins=[l_bnc.ap().opt()],
                outs=[gath2.ap().opt()],
            )
            g2 = small.tile([128, NCORES, 16], FP32, tag="g2")
            nc.sync.dma_start(g2[:], gath2.ap().rearrange("r p j -> p r j"), single_packet=True)

            # --- rank-sum (restores +gate_b') + sigmoid -> gainT [128, 16] ---
            lsum = small.tile([128, 16], FP32, tag="lsum")
            nc.vector.tensor_reduce(
                lsum[:], g2[:].rearrange("p r j -> p j r"), axis=AX.X, op=ALU.add
            )
            gainT = small.tile([128, 16], FP32, tag="gainT")
            nc.scalar.activation(gainT[:], lsum[:], AF.Sigmoid)

            # --- scale x in place (per-partition scalar) and stream out ---
            for j in range(NT):
                for q in range(4):
                    nc.vector.tensor_scalar_mul(
                        xs[j][:, q, :],
                        xs[j][:, q, :],
                        gainT[:, 4 * j + q : 4 * j + q + 1],
                    )
                    # first tile: store in halves so the write stream starts
                    # as early as possible after gain is known
                    if j == 0 and q == 1:
                        nc.sync.dma_start(out_d[j, :, 0:2, :], xs[j][:, 0:2, :])
                if j == 0:
                    nc.scalar.dma_start(out_d[j, :, 2:4, :], xs[j][:, 2:4, :])
                else:
                    # alternate the two HWDGE rings to keep all SDMA slots fed
                    eng = nc.sync if j % 2 == 0 else nc.scalar
                    eng.dma_start(out_d[j], xs[j][:])

    nc.compile()
    return nc


def _get_nc():
    if "nc" not in _NC_CACHE:
        _NC_CACHE["nc"] = _build()
    return _NC_CACHE["nc"]


def _shard_inputs(x, state, proj_w, proj_b, gate_w, gate_b):
    import ml_dtypes

    bf16 = ml_dtypes.bfloat16
    x = np.asarray(x, dtype=np.float32)
    state = np.asarray(state, dtype=np.float32)
    proj_w = np.asarray(proj_w, dtype=np.float32)
    proj_b = np.asarray(proj_b, dtype=np.float32)
    gate_w = np.asarray(gate_w, dtype=np.float32)
    gate_b = np.asarray(gate_b, dtype=np.float32)

    # EMA affine constant: ns = A*(proj_w/(B*T)) @ pooled_sum + eb, so the
    # x-independent gate logit part gate_w @ eb + gate_b folds into one bias.
    eb = DECAY * state + A * proj_b
    gb_eff = gate_w @ eb + gate_b
    # each core adds gb_eff/NCORES pre-AllGather; the rank-sum restores it
    gbt = np.ascontiguousarray(gb_eff.reshape(16, 128).T / float(NCORES))
    in_maps = []
    for c in range(NCORES):
        lo, hi = c * DS, (c + 1) * DS
        # xt[J, p, q, t] = x[c, t, 128*(4J+q)+p], bf16
        xt = np.ascontiguousarray(
            x[c].T.reshape(NT, 4, 128, T).transpose(0, 2, 1, 3).astype(bf16)
        )
        # ptw[p, m, kk, j] = (1-DECAY) * proj_w[lo+128m+j, 128kk+p] / (B*T)
        ptw = np.ascontiguousarray(
            (proj_w[lo:hi, :] * (A / float(B * T)))
            .reshape(2, 128, 16, 128)
            .transpose(3, 0, 2, 1)
            .astype(bf16)
        )
        # gtw[p, kk, jj, m] = gate_w[128jj+m, lo+128kk+p]
        gtw = np.ascontiguousarray(
            gate_w[:, lo:hi].reshape(16, 128, 2, 128).transpose(3, 2, 0, 1).astype(bf16)
        )
        in_maps.append({"xt": xt, "ptw": ptw, "gtw": gtw, "gbt": gbt})
    return in_maps


def _run(inputs, trace=False, **kwargs):
    nc = _get_nc()
    in_maps = _shard_inputs(**inputs)
    res = run_bass_kernel_spmd(
        nc, in_maps, core_ids=list(range(NCORES)), trace=trace, **kwargs
    )
    # out[c, t, 128*(2J+q)+p] = out_dev[J, p, q, t]
    outs = []
    for c in range(NCORES):
        od = res.results[c]["out"]  # [NT, 128, 2, T] bf16
        outs.append(
            od.transpose(0, 2, 1, 3).reshape(D, T).T.astype(np.float32)
        )
    out = np.ascontiguousarray(np.stack(outs, axis=0))
    return out, res


def kernel(**inputs):
    out, _ = _run(inputs, trace=False)
    return out
